# revision 48
# baseline (speedup 1.0000x reference)
"""Sparse dual-masked attention for Trainium2, 8 NeuronCores.

Problem: B=2, N=2048, DIM=512, H=8, DH=64.
  qkv = x @ W_qkv; per-head attention with dual mask
  (np_i*np_j==0 | bert_j==1 -> -1000), softmax, out proj + bias.

Structure exploited (sparse_attention):
  - A row i with np_i==0 is fully masked -> softmax uniform -> output row is
    the constant mean(V) @ W_out + b_out (computed on host; tiny).
  - For np_i==1 rows only columns with np_j==1 & bert_j==0 survive, so we
    gather those ~R=1030 rows / ~M=535 kv columns on the host and run dense
    attention over the gathered set on device (~8x less work than dense).

Sharding: core = (batch b, head-pair g): 2 batches x 4 head groups.
  W_qkv split column-wise per head pair, W_out row-wise; each core produces
  a partial [R,512] output; host sums the 4 partials per batch.

All matmul operands are bf16 (PSUM accumulation stays fp32): 1 cyc/row on
the PE at any free size vs 4 for fp32, half DMA/SBUF/LDWEIGHTS cost. A host
simulation of 8-bit-mantissa rounding through the whole pipeline gives
~2.6e-3 scale-relative error vs the 2e-2 gate.

Device dataflow per core (R_PAD query rows, M_PAD kv cols, 2 heads):
  xT [512, R_PAD] ships pre-gathered/transposed (kv rows first, then tail
  rows); kvc [128, NMT] is the kv-indicator column per m-tile.
  1. K^T = Wk^T x^T [128, M_PAD]; Q^T chunks computed lazily per r-chunk.
  2. Per r-chunk (PSUM-bank-sized, <=512), per m-tile: V_aug built lazily on
     the first pass: V rows scaled by kvc, plus kvc replicated into 64
     stationary columns per head so the attn@V matmul emits the softmax
     denominator REPLICATED on psum partitions 0:64 for free.
     S^T pair (both heads, disjoint PE row groups) -> one exp (ScalarE,
     2-bank PSUM AP -> bf16 SBUF) -> O^T += V_aug_h^T P_h^T accumulated
     over m-tiles into a 2-bank pair.
  3. recip = 1/denoms on the 64 replicated partitions directly (DVE approx),
     O^T normalized by tensor_mul -> OnT bf16; y = OnT^T @ W_out rows per
     128-r-tile as soon as both heads are normalized; DMA out interleaved.
  Host adds bias, sums the 4 head-group partials, fills masked rows.
"""

import numpy as np

_CORES = 8
_DIM = 512
_DH = 64
_H = 8
_INNER = _H * _DH


def _ceil_to(x, m):
    return ((x + m - 1) // m) * m


def _chunks(total, step):
    out = []
    o = 0
    while o < total:
        out.append((o, min(step, total - o)))
        o += step
    return out


def _chunks_ge(total, step=512, minc=256):
    """Chunks of <= step, each >= minc (rebalancing the tail)."""
    out = _chunks(total, step)
    if len(out) >= 2 and out[-1][1] < minc:
        o_prev, w_prev = out[-2]
        o_last, w_last = out[-1]
        move = minc - w_last
        out[-2] = (o_prev, w_prev - move)
        out[-1] = (o_last - move, w_last + move)
    return out


def build_bass(R_PAD, M_PAD):
    """Build the SPMD bass program for padded sizes R_PAD (queries) and
    M_PAD (kv columns). Returns the compiled Bacc object."""
    import concourse.bacc as bacc
    import concourse.mybir as mybir
    import concourse.tile as tile

    f32 = mybir.dt.float32
    bf16 = mybir.dt.bfloat16
    EXP = mybir.ActivationFunctionType.Exp

    assert R_PAD % 16 == 0 and M_PAD % 128 == 0 and R_PAD >= M_PAD
    NMT = M_PAD // 128          # kv m-tiles
    NRT = (R_PAD + 127) // 128  # query r-tiles for the final projection
    RC = _chunks_ge(R_PAD)      # r-chunks: one PSUM bank each, >=256
    MC = _chunks_ge(M_PAD)
    assert len(RC) <= 3

    nc = bacc.Bacc("TRN2", target_bir_lowering=False, debug=False,
                   num_devices=_CORES)

    xT_d = nc.dram_tensor("xT", [512, R_PAD], bf16, kind="ExternalInput")
    w3_d = nc.dram_tensor("w3", [512, 384], bf16, kind="ExternalInput")
    kvc_d = nc.dram_tensor("kvc", [128, NMT], f32, kind="ExternalInput")
    kvr_d = nc.dram_tensor("kvr", [128, NMT * 128], bf16,
                           kind="ExternalInput")
    wo_d = nc.dram_tensor("wo", [128, 512], bf16, kind="ExternalInput")
    y_d = nc.dram_tensor("y", [R_PAD, 512], bf16, kind="ExternalOutput")

    with tile.TileContext(nc) as tc:
        with (
            tc.tile_pool(name="consts", bufs=1) as consts,
            tc.tile_pool(name="pt", bufs=4) as ptpool,
            tc.tile_pool(name="rcp", bufs=2) as rpool,
            tc.tile_pool(name="ysb", bufs=4) as ypool,
            tc.tile_pool(name="psS", bufs=2, space="PSUM") as psS,
            tc.tile_pool(name="psO", bufs=1, space="PSUM") as psO,
            tc.tile_pool(name="psQ", bufs=1, space="PSUM") as psQ,
        ):
            # ---- input DMAs: merged to 8 issues over the 3 DMA queues;
            # the critical first wave (all weights + xT's kv column
            # prefix) is balanced one-per-queue --------------------------
            w3 = consts.tile([128, 4, 384], bf16, tag="w3")
            nc.scalar.dma_start(
                out=w3, in_=w3_d.ap().rearrange("(a p) d -> p a d", p=128))
            xT = consts.tile([128, 4, R_PAD], bf16, tag="xT")
            Vt = consts.tile([128, NMT, 2, 128], bf16, tag="Vt")
            for cp, eng in ((0, nc.sync), (1, nc.gpsimd)):
                nc_sl = slice(cp * 256, cp * 256 + 256)
                eng.dma_start(
                    out=xT[:, 2 * cp:2 * cp + 2, 0:M_PAD],
                    in_=xT_d.ap()[nc_sl, 0:M_PAD].rearrange(
                        "(a p) d -> p a d", p=128))
            kvc = consts.tile([128, NMT], f32, tag="kvc")
            wo = consts.tile([128, 512], bf16, tag="wo")
            for cp, eng in ((0, nc.sync), (1, nc.gpsimd)):
                nc_sl = slice(cp * 256, cp * 256 + 256)
                eng.dma_start(
                    out=xT[:, 2 * cp:2 * cp + 2, M_PAD:R_PAD],
                    in_=xT_d.ap()[nc_sl, M_PAD:R_PAD].rearrange(
                        "(a p) d -> p a d", p=128))
            nc.scalar.dma_start(out=kvc, in_=kvc_d.ap())
            nc.scalar.dma_start(
                out=Vt[:, :, :, 0:64],
                in_=kvr_d.ap().rearrange("p (a b c) -> p a b c", b=2, c=64))
            nc.scalar.dma_start(out=wo, in_=wo_d.ap())
            def wq(c):
                return w3[:, c, 0:128]

            def wk(c):
                return w3[:, c, 128:256]

            def wv(c, h):
                return w3[:, c, 256 + h * 64:256 + (h + 1) * 64]

            # ---- K projection chunk 0 (chunk 1 is deferred into the ci0
            # loop; Q is computed lazily per r-chunk below) ---------------
            KT = consts.tile([128, M_PAD], bf16, tag="KT")

            def k_proj(i):
                o, w = MC[i]
                # chunk 0 precedes the op(ci0) allocation so tag O is
                # safe; chunk 1 is emitted mid-ci0 and must avoid tag O
                # (it would WAR-deadlock against the live O accumulator).
                pool, tag = (psO, "O") if i == 0 else (psQ, "Q")
                ps = pool.tile([128, 2, 512], f32, tag=tag, name=f"kps{i}")
                for c in range(4):
                    nc.tensor.matmul(ps[:, 0, :w], wk(c),
                                     xT[:, c, o:o + w],
                                     start=(c == 0), stop=(c == 3))
                if i == 0:
                    # piecewise: the first 128 columns unblock S(mt0)
                    # ~0.3us earlier than a single full-width copy.
                    nc.scalar.copy(KT[:, o:o + 128], ps[:, 0, :128])
                    nc.scalar.copy(KT[:, o + 128:o + w], ps[:, 0, 128:w])
                else:
                    nc.vector.tensor_copy(KT[:, o:o + w], ps[:, 0, :w])

            k_proj(0)

            # ---- main loop: per r-chunk, per m-tile: S pair -> exp ->
            # O accumulate; V_aug built lazily on the first r-chunk --------
            QT = consts.tile([128, R_PAD], bf16, tag="QT")
            OnT = consts.tile([128, R_PAD], bf16, tag="OnT")
            # y r-tile groups: consecutive FULL tiles pair up; a partial
            # trailing tile is emitted alone.
            n_full = R_PAD // 128
            YG = [(i, i + 1) for i in range(0, n_full - 1, 2)]
            if n_full % 2 == 1:
                YG.append((n_full - 1,))
            if R_PAD % 128 != 0:
                YG.append((n_full,))
            ydone = 0

            def q_proj(ci):
                o, w = RC[ci]
                qps = psQ.tile([128, 2, 512], f32, tag="Q", name=f"qps{ci}")
                for c in range(4):
                    nc.tensor.matmul(qps[:, 0, :w], wq(c),
                                     xT[:, c, o:o + w],
                                     start=(c == 0), stop=(c == 3))
                nc.vector.tensor_copy(QT[:, o:o + w], qps[:, 0, :w])

            def emit_y(rts, k, tail=True):
                lo = rts[0] * 128
                hi = min(rts[-1] * 128 + 128, R_PAD)
                yp = psQ.tile([128, 2, 512], f32, tag="Q",
                              name=f"yp{rts[0]}")
                for j, rt in enumerate(rts):
                    tw = min(128, R_PAD - rt * 128)
                    nc.tensor.matmul(
                        yp[:tw, j, :], OnT[:, rt * 128:rt * 128 + tw],
                        wo, start=True, stop=True)
                # mid-loop copies go to DVE (an ACT copy would delay the
                # exp stream); tail copies alternate so they run in
                # parallel across both engines.
                copy = (nc.scalar.copy if tail and k % 2 == 1
                        else nc.vector.tensor_copy)
                deng = [nc.sync, nc.scalar, nc.gpsimd][k % 3]
                ysb = ypool.tile([128, 2, 512], bf16, tag="y")
                if len(rts) == 2:
                    copy(ysb, yp)
                    deng.dma_start(
                        out=y_d.ap()[lo:hi, :].rearrange(
                            "(a p) d -> p a d", p=128),
                        in_=ysb)
                else:
                    copy(ysb[:hi - lo, 0, :], yp[:hi - lo, 0, :])
                    deng.dma_start(out=y_d.ap()[lo:hi, :],
                                   in_=ysb[:hi - lo, 0, :])

            pending = []
            nemit = 0
            pts_all = [dict() for _ in RC]

            def s_exp(ci, mt):
                o, w = RC[ci]
                msl = slice(mt * 128, (mt + 1) * 128)
                sp = psS.tile([128, 2, 512], f32, tag="S",
                              name=f"sp{ci}_{mt}")
                for h in range(2):
                    hs = slice(h * 64, (h + 1) * 64)
                    nc.tensor.matmul(sp[:, h, :w], KT[hs, msl],
                                     QT[hs, o:o + w],
                                     start=True, stop=True)
                pt = ptpool.tile([128, 2, 512], bf16, tag="pt",
                                 name=f"pt{ci}_{mt}")
                nc.scalar.activation(out=pt[:, :, :w], in_=sp[:, :, :w],
                                     func=EXP)
                pts_all[ci][mt] = pt

            q_proj(0)
            s_done = [0] * len(RC)
            for ci, (o, w) in enumerate(RC):
                op = psO.tile([128, 2, 512], f32, tag="O", name=f"op{ci}")
                pts = pts_all[ci]
                for mt in range(NMT):
                    if s_done[ci] <= mt:
                        s_exp(ci, mt)
                        s_done[ci] = mt + 1
                    # everything below is emitted AFTER the exp so the
                    # S(mt+1) -> exp(mt+1) critical chain never queues
                    # behind projection / output work on the PE.
                    if mt == 2 and ci + 1 < len(RC):
                        # next r-chunk's Q projection, early enough that
                        # its QT copy clears the DVE queue before this
                        # chunk's rcp/muls and y copies pile in, and
                        # before the y groups grab the psQ slot.
                        q_proj(ci + 1)
                    if mt >= 1 and pending:
                        # deferred y group from the previous r-chunk: by
                        # now its OnT muls are long done, so the PE queue
                        # doesn't stall on them.
                        emit_y(pending.pop(0), nemit, tail=False)
                        nemit += 1
                    if ci == 0:
                        # lazy V_aug into Vt[:, mt] (AFTER S/exp so the
                        # first exps aren't queued behind V work):
                        # [kvc x64 | V_h x64] per head; the kvc columns
                        # (DMA'd from host) make the O matmul emit the
                        # softmax denominator replicated on partitions
                        # 0:64 and null the tail rows below M_PAD.
                        msl = slice(mt * 128, (mt + 1) * 128)
                        vps = psQ.tile([128, 2, 512], f32, tag="Q",
                                       name=f"vps{mt}")
                        for c in range(4):
                            nc.tensor.matmul(
                                vps[:, 0, 0:128], xT[:, c, msl],
                                w3[:, c, 256:384],
                                start=(c == 0), stop=(c == 3))
                        nc.vector.tensor_scalar_mul(
                            Vt[:, mt, :, 64:128],
                            in0=vps[:, 0, 0:128].rearrange(
                                "p (a b) -> p a b", a=2),
                            scalar1=kvc[:, mt:mt + 1])
                        if mt == 1:
                            k_proj(1)
                    # O for the PREVIOUS m-tile: keeps the PE busy on
                    # S(mt) while ScalarE runs exp(mt-1) instead of the
                    # in-order PE queue stalling on exp(mt).
                    if mt > 0:
                        for h in range(2):
                            nc.tensor.matmul(op[:, h, :w],
                                             Vt[:, mt - 1, h, :],
                                             pts[mt - 1][:, h, :w],
                                             start=(mt == 1), stop=False)
                if ci + 1 < len(RC):
                    # peel the next chunk's first S+exp ahead of this
                    # chunk's epilogue so the exp stream never drains at
                    # the chunk boundary.
                    s_exp(ci + 1, 0)
                    s_done[ci + 1] = 1
                for h in range(2):
                    nc.tensor.matmul(op[:, h, :w], Vt[:, NMT - 1, h, :],
                                     pts[NMT - 1][:, h, :w],
                                     start=False, stop=True)

                # normalize: denominators sit replicated on partitions
                # 0:64. For the FINAL chunk, normalize per y-group column
                # range and emit each group as soon as its columns are
                # ready, shortening the serial tail.
                done = o + w
                grps = []
                while (ydone < len(YG) and
                       min(YG[ydone][-1] * 128 + 128, R_PAD) <= done):
                    grps.append(YG[ydone])
                    ydone += 1
                last = ci == len(RC) - 1
                aligned = all(g[0] * 128 >= o and
                              min(g[-1] * 128 + 128, R_PAD) <= o + w
                              for g in grps)
                if last and aligned and not pending:
                    for g in grps:
                        glo = g[0] * 128
                        ghi = min(g[-1] * 128 + 128, R_PAD)
                        gw = ghi - glo
                        rc = rpool.tile([64, 2, 512], f32, tag="rcp",
                                        name=f"rc{ci}_{g[0]}")
                        nc.vector.reciprocal_approx_fast(
                            rc[:, :, :gw], op[0:64, :, glo - o:ghi - o])
                        for h in range(2):
                            nc.vector.tensor_mul(
                                OnT[h * 64:(h + 1) * 64, glo:ghi],
                                op[64:128, h, glo - o:ghi - o],
                                rc[:, h, :gw])
                        emit_y(g, nemit)
                        nemit += 1
                else:
                    rc = rpool.tile([64, 2, 512], f32, tag="rcp",
                                    name=f"rc{ci}")
                    nc.vector.reciprocal_approx_fast(rc[:, :, :w],
                                                     op[0:64, :, :w])
                    for h in range(2):
                        nc.vector.tensor_mul(
                            OnT[h * 64:(h + 1) * 64, o:o + w],
                            op[64:128, h, :w], rc[:, h, :w])
                    pending.extend(grps)
            for rts in pending:
                emit_y(rts, nemit)
                nemit += 1

    nc.compile()
    return nc


def _prep(x, mask_np, mask_bert, W_qkv, W_out):
    """Host-side gather/shard. Returns (in_maps, meta)."""
    import ml_dtypes
    bf16 = ml_dtypes.bfloat16

    B, N, DIM = x.shape
    assert (B, DIM) == (2, _DIM)
    x = np.ascontiguousarray(x, dtype=np.float32)
    W_qkv = np.ascontiguousarray(W_qkv, dtype=np.float32)
    W_out = np.ascontiguousarray(W_out, dtype=np.float32)

    kv_idx, tail_idx, Ms, tails = [], [], [], []
    for b in range(B):
        npb = mask_np[b].astype(bool)
        bb = mask_bert[b].astype(bool)
        kv = np.nonzero(npb & ~bb)[0]
        tl = np.nonzero(npb & bb)[0]
        kv_idx.append(kv)
        tail_idx.append(tl)
        Ms.append(len(kv))
        tails.append(len(tl))

    M_PAD = max(128, _ceil_to(max(Ms), 128))
    # Cap device rows at R_CAP (PSUM-bank-aligned chunks, fewer exps);
    # tail rows beyond the cap are queries only (never keys), so they are
    # peeled off and computed exactly on the host (a handful of rows).
    R_CAP = max(1024, M_PAD)
    over_idx = []
    for b in range(B):
        n_over = max(0, Ms[b] + tails[b] - R_CAP)
        over_idx.append(tail_idx[b][tails[b] - n_over:])
        tail_idx[b] = tail_idx[b][:tails[b] - n_over]
        tails[b] -= n_over
    # rows are packed [kv | tail] with no gap: the tail rows that fall in
    # [M_b, M_PAD) act as key/value candidates but are nulled by the kvc
    # indicator (V rows scaled to 0, denominator columns 0).
    R_PAD = max(128, _ceil_to(max(Ms[b] + tails[b] for b in range(B)), 16),
                M_PAD)

    NMT = M_PAD // 128
    xT_b, kvc_b, kvr_b, row_pos = [], [], [], []
    for b in range(B):
        xa = np.zeros((512, R_PAD), dtype=np.float32)
        xa[:, :Ms[b]] = x[b][kv_idx[b]].T
        xa[:, Ms[b]:Ms[b] + tails[b]] = x[b][tail_idx[b]].T
        xT_b.append(np.ascontiguousarray(xa.astype(bf16)))
        kvones = np.zeros(M_PAD, dtype=np.float32)
        kvones[:Ms[b]] = 1.0
        kvc_b.append(np.ascontiguousarray(kvones.reshape(NMT, 128).T))
        # kvc replicated into the 64 denominator columns per (m-tile, head)
        kvr = np.broadcast_to(kvc_b[b][:, :, None, None],
                              (128, NMT, 2, 64))
        kvr_b.append(np.ascontiguousarray(
            kvr.reshape(128, NMT * 128).astype(bf16)))
        # output row p of the device result corresponds to token row_pos[p]
        pos = np.concatenate([kv_idx[b], tail_idx[b]])
        row_pos.append(pos)

    scale = np.float32(_DH ** -0.5)
    in_maps = []
    for c in range(_CORES):
        b, g = divmod(c, 4)
        qc = slice(128 * g, 128 * g + 128)
        kc = slice(_INNER + 128 * g, _INNER + 128 * g + 128)
        vc = slice(2 * _INNER + 128 * g, 2 * _INNER + 128 * g + 128)
        w3 = np.ascontiguousarray(np.concatenate(
            [W_qkv[:, qc] * scale, W_qkv[:, kc], W_qkv[:, vc]],
            axis=1).astype(bf16))
        wo = np.ascontiguousarray(
            W_out[128 * g:128 * g + 128, :].astype(bf16))
        in_maps.append({"xT": xT_b[b], "w3": w3, "wo": wo,
                        "kvc": kvc_b[b], "kvr": kvr_b[b]})

    meta = dict(M_PAD=M_PAD, R_PAD=R_PAD, Ms=Ms, tails=tails,
                kv_idx=kv_idx, tail_idx=tail_idx, row_pos=row_pos,
                over_idx=over_idx)
    return in_maps, meta


def _assemble(results, meta, x, mask_np, W_qkv, W_out, b_out):
    B, N, _ = x.shape
    out = np.empty((B, N, _DIM), dtype=np.float32)
    Wv_full = W_qkv[:, 2 * _INNER:].astype(np.float32)
    for b in range(B):
        # constant output for fully-masked rows: uniform attention = mean(V)
        meanv = (x[b].mean(axis=0, dtype=np.float32) @ Wv_full)
        yconst = meanv @ W_out.astype(np.float32) + b_out
        out[b, :, :] = yconst[None, :]
        Mb, tb = meta["Ms"][b], meta["tails"][b]
        if Mb == 0:
            # no unmasked kv columns: every row fully masked -> uniform
            continue
        acc = None
        for g in range(4):
            yp = results[4 * b + g]["y"].astype(np.float32)
            acc = yp if acc is None else acc + yp
        out[b, meta["row_pos"][b], :] = acc[:Mb + tb] + b_out
        ov = meta["over_idx"][b]
        if len(ov):
            # host-peeled tail rows (queries only): exact attention over
            # the surviving kv columns.
            kv = meta["kv_idx"][b]
            scale = np.float32(_DH ** -0.5)
            q = (x[b][ov].astype(np.float32) @
                 W_qkv[:, :_INNER]).reshape(len(ov), _H, _DH)
            k = (x[b][kv].astype(np.float32) @
                 W_qkv[:, _INNER:2 * _INNER]).reshape(Mb, _H, _DH)
            v = (x[b][kv].astype(np.float32) @
                 W_qkv[:, 2 * _INNER:]).reshape(Mb, _H, _DH)
            s = np.einsum('ihd,jhd->hij', q, k) * scale
            s -= s.max(axis=-1, keepdims=True)
            p = np.exp(s)
            p /= p.sum(axis=-1, keepdims=True)
            o = np.einsum('hij,jhd->ihd', p, v).reshape(len(ov), _INNER)
            out[b, ov, :] = o @ W_out + b_out
    return out


_CACHE = {}


def _get_bass(R_PAD, M_PAD):
    key = (R_PAD, M_PAD)
    if key not in _CACHE:
        _CACHE[key] = build_bass(R_PAD, M_PAD)
    return _CACHE[key]


def run_spmd(in_maps, meta, trace=False, tmpdir=None, trace_cores=None):
    from concourse.bass_utils import run_bass_kernel_spmd

    nc = _get_bass(meta["R_PAD"], meta["M_PAD"])
    return run_bass_kernel_spmd(
        nc, in_maps, core_ids=list(range(_CORES)), trace=trace, tmpdir=tmpdir,
        trace_cores=trace_cores)


def kernel(x, mask_np, mask_bert, W_qkv, W_out, b_out):
    x = np.asarray(x)
    mask_np = np.asarray(mask_np)
    mask_bert = np.asarray(mask_bert)
    W_qkv = np.asarray(W_qkv, dtype=np.float32)
    W_out = np.asarray(W_out, dtype=np.float32)
    b_out = np.asarray(b_out, dtype=np.float32)

    in_maps, meta = _prep(x, mask_np, mask_bert, W_qkv, W_out)
    res = run_spmd(in_maps, meta)
    return _assemble(res.results, meta, x, mask_np, W_qkv, W_out, b_out)


# revision 49
# speedup vs baseline: 1.1986x; 1.1986x over previous
"""Sparse dual-masked attention for Trainium2, 8 NeuronCores.

Problem: B=2, N=2048, DIM=512, H=8, DH=64.
  qkv = x @ W_qkv; per-head attention with dual mask
  (np_i*np_j==0 | bert_j==1 -> -1000), softmax, out proj + bias.

Structure exploited (sparse_attention):
  - A row i with np_i==0 is fully masked -> softmax uniform -> output row is
    the constant mean(V) @ W_out + b_out (computed on host; tiny).
  - For np_i==1 rows only columns with np_j==1 & bert_j==0 survive, so we
    gather those ~R=1030 rows / ~M=535 kv columns on the host and run dense
    attention over the gathered set on device (~8x less work than dense).

Sharding: core = (batch b, head-pair g): 2 batches x 4 head groups.
  W_qkv split column-wise per head pair, W_out row-wise; each core produces
  a partial [R,512] output; host sums the 4 partials per batch.

All matmul operands are bf16 (PSUM accumulation stays fp32): 1 cyc/row on
the PE at any free size vs 4 for fp32, half DMA/SBUF/LDWEIGHTS cost. A host
simulation of 8-bit-mantissa rounding through the whole pipeline gives
~2.6e-3 scale-relative error vs the 2e-2 gate.

Device dataflow per core (R_PAD query rows, M_PAD kv cols, 2 heads):
  xT [512, R_PAD] ships pre-gathered/transposed (kv rows first, then tail
  rows); kvc [128, NMT] is the kv-indicator column per m-tile.
  1. K^T = Wk^T x^T [128, M_PAD]; Q^T chunks computed lazily per r-chunk.
  2. Per r-chunk (PSUM-bank-sized, <=512), per m-tile: V_aug built lazily on
     the first pass: V rows scaled by kvc, plus kvc replicated into 64
     stationary columns per head so the attn@V matmul emits the softmax
     denominator REPLICATED on psum partitions 0:64 for free.
     S^T pair (both heads, disjoint PE row groups) -> one exp (ScalarE,
     2-bank PSUM AP -> bf16 SBUF) -> O^T += V_aug_h^T P_h^T accumulated
     over m-tiles into a 2-bank pair.
  3. recip = 1/denoms on the 64 replicated partitions directly (DVE approx),
     O^T normalized by tensor_mul -> OnT bf16; y = OnT^T @ W_out rows per
     128-r-tile as soon as both heads are normalized; DMA out interleaved.
  Host adds bias, sums the 4 head-group partials, fills masked rows.
"""

import numpy as np

_CORES = 8
_DIM = 512
_DH = 64
_H = 8
_INNER = _H * _DH


def _ceil_to(x, m):
    return ((x + m - 1) // m) * m


def _chunks(total, step):
    out = []
    o = 0
    while o < total:
        out.append((o, min(step, total - o)))
        o += step
    return out


def _chunks_ge(total, step=512, minc=256):
    """Chunks of <= step, each >= minc (rebalancing the tail)."""
    out = _chunks(total, step)
    if len(out) >= 2 and out[-1][1] < minc:
        o_prev, w_prev = out[-2]
        o_last, w_last = out[-1]
        move = minc - w_last
        out[-2] = (o_prev, w_prev - move)
        out[-1] = (o_last - move, w_last + move)
    return out


def build_bass(R_PAD, M_PAD):
    """Build the SPMD bass program for padded sizes R_PAD (queries) and
    M_PAD (kv columns). Returns the compiled Bacc object."""
    import concourse.bacc as bacc
    import concourse.mybir as mybir
    import concourse.tile as tile

    f32 = mybir.dt.float32
    bf16 = mybir.dt.bfloat16
    EXP = mybir.ActivationFunctionType.Exp

    assert R_PAD % 16 == 0 and M_PAD % 128 == 0 and R_PAD >= M_PAD
    NMT = M_PAD // 128          # kv m-tiles
    NRT = (R_PAD + 127) // 128  # query r-tiles for the final projection
    RC = _chunks_ge(R_PAD)      # r-chunks: one PSUM bank each, >=256
    MC = _chunks_ge(M_PAD)
    assert len(RC) <= 3

    nc = bacc.Bacc("TRN2", target_bir_lowering=False, debug=False,
                   num_devices=_CORES)

    xT_d = nc.dram_tensor("xT", [512, R_PAD], bf16, kind="ExternalInput")
    w3_d = nc.dram_tensor("w3", [512, 384], bf16, kind="ExternalInput")
    kvc_d = nc.dram_tensor("kvc", [128, NMT], f32, kind="ExternalInput")
    kvr_d = nc.dram_tensor("kvr", [128, NMT * 128], bf16,
                           kind="ExternalInput")
    wo_d = nc.dram_tensor("wo", [128, 512], bf16, kind="ExternalInput")
    y_d = nc.dram_tensor("y", [R_PAD, 512], bf16, kind="ExternalOutput")

    with tile.TileContext(nc) as tc:
        with (
            tc.tile_pool(name="consts", bufs=1) as consts,
            tc.tile_pool(name="pt", bufs=4) as ptpool,
            tc.tile_pool(name="rcp", bufs=2) as rpool,
            tc.tile_pool(name="ysb", bufs=4) as ypool,
            tc.tile_pool(name="psS", bufs=2, space="PSUM") as psS,
            tc.tile_pool(name="psO", bufs=1, space="PSUM") as psO,
            tc.tile_pool(name="psQ", bufs=1, space="PSUM") as psQ,
        ):
            # ---- input DMAs: merged to 8 issues over the 3 DMA queues;
            # the critical first wave (all weights + xT's kv column
            # prefix) is balanced one-per-queue --------------------------
            w3 = consts.tile([128, 4, 384], bf16, tag="w3")
            nc.scalar.dma_start(
                out=w3, in_=w3_d.ap().rearrange("(a p) d -> p a d", p=128))
            xT = consts.tile([128, 4, R_PAD], bf16, tag="xT")
            Vt = consts.tile([128, NMT, 2, 128], bf16, tag="Vt")
            for cp, eng in ((0, nc.sync), (1, nc.gpsimd)):
                nc_sl = slice(cp * 256, cp * 256 + 256)
                eng.dma_start(
                    out=xT[:, 2 * cp:2 * cp + 2, 0:M_PAD],
                    in_=xT_d.ap()[nc_sl, 0:M_PAD].rearrange(
                        "(a p) d -> p a d", p=128))
            kvc = consts.tile([128, NMT], f32, tag="kvc")
            wo = consts.tile([128, 512], bf16, tag="wo")
            for cp, eng in ((0, nc.sync), (1, nc.gpsimd)):
                nc_sl = slice(cp * 256, cp * 256 + 256)
                eng.dma_start(
                    out=xT[:, 2 * cp:2 * cp + 2, M_PAD:R_PAD],
                    in_=xT_d.ap()[nc_sl, M_PAD:R_PAD].rearrange(
                        "(a p) d -> p a d", p=128))
            nc.scalar.dma_start(out=kvc, in_=kvc_d.ap())
            nc.scalar.dma_start(
                out=Vt[:, :, :, 0:64],
                in_=kvr_d.ap().rearrange("p (a b c) -> p a b c", b=2, c=64))
            nc.scalar.dma_start(out=wo, in_=wo_d.ap())
            def wq(c):
                return w3[:, c, 0:128]

            def wk(c):
                return w3[:, c, 128:256]

            def wv(c, h):
                return w3[:, c, 256 + h * 64:256 + (h + 1) * 64]

            # ---- K projection chunk 0 (chunk 1 is deferred into the ci0
            # loop; Q is computed lazily per r-chunk below) ---------------
            KT = consts.tile([128, M_PAD], bf16, tag="KT")

            def k_proj(i):
                o, w = MC[i]
                # chunk 0 precedes the op(ci0) allocation so tag O is
                # safe; chunk 1 is emitted mid-ci0 and must avoid tag O
                # (it would WAR-deadlock against the live O accumulator).
                pool, tag = (psO, "O") if i == 0 else (psQ, "Q")
                ps = pool.tile([128, 2, 512], f32, tag=tag, name=f"kps{i}")
                for c in range(4):
                    nc.tensor.matmul(ps[:, 0, :w], wk(c),
                                     xT[:, c, o:o + w],
                                     start=(c == 0), stop=(c == 3))
                if i == 0:
                    # piecewise: the first 128 columns unblock S(mt0)
                    # ~0.3us earlier than a single full-width copy.
                    nc.scalar.copy(KT[:, o:o + 128], ps[:, 0, :128])
                    nc.scalar.copy(KT[:, o + 128:o + w], ps[:, 0, 128:w])
                else:
                    nc.vector.tensor_copy(KT[:, o:o + w], ps[:, 0, :w])

            k_proj(0)

            # ---- main loop: per r-chunk, per m-tile: S pair -> exp ->
            # O accumulate; V_aug built lazily on the first r-chunk --------
            QT = consts.tile([128, R_PAD], bf16, tag="QT")
            OnT = consts.tile([128, R_PAD], bf16, tag="OnT")
            # y r-tile groups: consecutive FULL tiles pair up; a partial
            # trailing tile is emitted alone.
            n_full = R_PAD // 128
            YG = [(i, i + 1) for i in range(0, n_full - 1, 2)]
            if n_full % 2 == 1:
                YG.append((n_full - 1,))
            if R_PAD % 128 != 0:
                YG.append((n_full,))
            ydone = 0

            def q_proj(ci):
                o, w = RC[ci]
                qps = psQ.tile([128, 2, 512], f32, tag="Q", name=f"qps{ci}")
                for c in range(4):
                    nc.tensor.matmul(qps[:, 0, :w], wq(c),
                                     xT[:, c, o:o + w],
                                     start=(c == 0), stop=(c == 3))
                nc.vector.tensor_copy(QT[:, o:o + w], qps[:, 0, :w])

            def emit_y(rts, k, tail=True):
                lo = rts[0] * 128
                hi = min(rts[-1] * 128 + 128, R_PAD)
                yp = psQ.tile([128, 2, 512], f32, tag="Q",
                              name=f"yp{rts[0]}")
                for j, rt in enumerate(rts):
                    tw = min(128, R_PAD - rt * 128)
                    nc.tensor.matmul(
                        yp[:tw, j, :], OnT[:, rt * 128:rt * 128 + tw],
                        wo, start=True, stop=True)
                # mid-loop copies go to DVE (an ACT copy would delay the
                # exp stream); tail copies alternate so they run in
                # parallel across both engines.
                copy = (nc.scalar.copy if tail and k % 2 == 1
                        else nc.vector.tensor_copy)
                deng = [nc.sync, nc.scalar, nc.gpsimd][k % 3]
                ysb = ypool.tile([128, 2, 512], bf16, tag="y")
                if len(rts) == 2:
                    copy(ysb, yp)
                    deng.dma_start(
                        out=y_d.ap()[lo:hi, :].rearrange(
                            "(a p) d -> p a d", p=128),
                        in_=ysb)
                else:
                    copy(ysb[:hi - lo, 0, :], yp[:hi - lo, 0, :])
                    deng.dma_start(out=y_d.ap()[lo:hi, :],
                                   in_=ysb[:hi - lo, 0, :])

            pending = []
            nemit = 0
            pts_all = [dict() for _ in RC]

            def s_exp(ci, mt):
                o, w = RC[ci]
                msl = slice(mt * 128, (mt + 1) * 128)
                sp = psS.tile([128, 2, 512], f32, tag="S",
                              name=f"sp{ci}_{mt}")
                for h in range(2):
                    hs = slice(h * 64, (h + 1) * 64)
                    nc.tensor.matmul(sp[:, h, :w], KT[hs, msl],
                                     QT[hs, o:o + w],
                                     start=True, stop=True)
                pt = ptpool.tile([128, 2, 512], bf16, tag="pt",
                                 name=f"pt{ci}_{mt}")
                nc.scalar.activation(out=pt[:, :, :w], in_=sp[:, :, :w],
                                     func=EXP)
                pts_all[ci][mt] = pt

            q_proj(0)
            s_done = [0] * len(RC)
            for ci, (o, w) in enumerate(RC):
                op = psO.tile([128, 2, 512], f32, tag="O", name=f"op{ci}")
                pts = pts_all[ci]
                for mt in range(NMT):
                    if s_done[ci] <= mt:
                        s_exp(ci, mt)
                        s_done[ci] = mt + 1
                    # everything below is emitted AFTER the exp so the
                    # S(mt+1) -> exp(mt+1) critical chain never queues
                    # behind projection / output work on the PE.
                    if mt == 2 and ci + 1 < len(RC):
                        # next r-chunk's Q projection, early enough that
                        # its QT copy clears the DVE queue before this
                        # chunk's rcp/muls and y copies pile in, and
                        # before the y groups grab the psQ slot.
                        q_proj(ci + 1)
                    if mt >= 1 and pending:
                        # deferred y group from the previous r-chunk: by
                        # now its OnT muls are long done, so the PE queue
                        # doesn't stall on them.
                        emit_y(pending.pop(0), nemit, tail=False)
                        nemit += 1
                    if ci == 0:
                        # lazy V_aug into Vt[:, mt] (AFTER S/exp so the
                        # first exps aren't queued behind V work):
                        # [kvc x64 | V_h x64] per head; the kvc columns
                        # (DMA'd from host) make the O matmul emit the
                        # softmax denominator replicated on partitions
                        # 0:64 and null the tail rows below M_PAD.
                        msl = slice(mt * 128, (mt + 1) * 128)
                        vps = psQ.tile([128, 2, 512], f32, tag="Q",
                                       name=f"vps{mt}")
                        for c in range(4):
                            nc.tensor.matmul(
                                vps[:, 0, 0:128], xT[:, c, msl],
                                w3[:, c, 256:384],
                                start=(c == 0), stop=(c == 3))
                        nc.vector.tensor_scalar_mul(
                            Vt[:, mt, :, 64:128],
                            in0=vps[:, 0, 0:128].rearrange(
                                "p (a b) -> p a b", a=2),
                            scalar1=kvc[:, mt:mt + 1])
                        if mt == 1:
                            k_proj(1)
                    # O for the PREVIOUS m-tile: keeps the PE busy on
                    # S(mt) while ScalarE runs exp(mt-1) instead of the
                    # in-order PE queue stalling on exp(mt).
                    if mt > 0:
                        for h in range(2):
                            nc.tensor.matmul(op[:, h, :w],
                                             Vt[:, mt - 1, h, :],
                                             pts[mt - 1][:, h, :w],
                                             start=(mt == 1), stop=False)
                for h in range(2):
                    nc.tensor.matmul(op[:, h, :w], Vt[:, NMT - 1, h, :],
                                     pts[NMT - 1][:, h, :w],
                                     start=False, stop=True)

                # normalize: denominators sit replicated on partitions
                # 0:64. For the FINAL chunk, normalize per y-group column
                # range and emit each group as soon as its columns are
                # ready, shortening the serial tail.
                done = o + w
                grps = []
                while (ydone < len(YG) and
                       min(YG[ydone][-1] * 128 + 128, R_PAD) <= done):
                    grps.append(YG[ydone])
                    ydone += 1
                last = ci == len(RC) - 1
                aligned = all(g[0] * 128 >= o and
                              min(g[-1] * 128 + 128, R_PAD) <= o + w
                              for g in grps)
                if last and aligned and not pending:
                    for g in grps:
                        glo = g[0] * 128
                        ghi = min(g[-1] * 128 + 128, R_PAD)
                        gw = ghi - glo
                        rc = rpool.tile([64, 2, 512], f32, tag="rcp",
                                        name=f"rc{ci}_{g[0]}")
                        nc.vector.reciprocal_approx_fast(
                            rc[:, :, :gw], op[0:64, :, glo - o:ghi - o])
                        for h in range(2):
                            nc.vector.tensor_mul(
                                OnT[h * 64:(h + 1) * 64, glo:ghi],
                                op[64:128, h, glo - o:ghi - o],
                                rc[:, h, :gw])
                        emit_y(g, nemit)
                        nemit += 1
                else:
                    rc = rpool.tile([64, 2, 512], f32, tag="rcp",
                                    name=f"rc{ci}")
                    nc.vector.reciprocal_approx_fast(rc[:, :, :w],
                                                     op[0:64, :, :w])
                    for h in range(2):
                        nc.vector.tensor_mul(
                            OnT[h * 64:(h + 1) * 64, o:o + w],
                            op[64:128, h, :w], rc[:, h, :w])
                    pending.extend(grps)
            for rts in pending:
                emit_y(rts, nemit)
                nemit += 1

    nc.compile()
    return nc


def _prep(x, mask_np, mask_bert, W_qkv, W_out):
    """Host-side gather/shard. Returns (in_maps, meta)."""
    import ml_dtypes
    bf16 = ml_dtypes.bfloat16

    B, N, DIM = x.shape
    assert (B, DIM) == (2, _DIM)
    x = np.ascontiguousarray(x, dtype=np.float32)
    W_qkv = np.ascontiguousarray(W_qkv, dtype=np.float32)
    W_out = np.ascontiguousarray(W_out, dtype=np.float32)

    kv_idx, tail_idx, Ms, tails = [], [], [], []
    for b in range(B):
        npb = mask_np[b].astype(bool)
        bb = mask_bert[b].astype(bool)
        kv = np.nonzero(npb & ~bb)[0]
        tl = np.nonzero(npb & bb)[0]
        kv_idx.append(kv)
        tail_idx.append(tl)
        Ms.append(len(kv))
        tails.append(len(tl))

    M_PAD = max(128, _ceil_to(max(Ms), 128))
    # Cap device rows at R_CAP (PSUM-bank-aligned chunks, fewer exps);
    # tail rows beyond the cap are queries only (never keys), so they are
    # peeled off and computed exactly on the host (a handful of rows).
    R_CAP = max(1024, M_PAD)
    over_idx = []
    for b in range(B):
        n_over = max(0, Ms[b] + tails[b] - R_CAP)
        over_idx.append(tail_idx[b][tails[b] - n_over:])
        tail_idx[b] = tail_idx[b][:tails[b] - n_over]
        tails[b] -= n_over
    # rows are packed [kv | tail] with no gap: the tail rows that fall in
    # [M_b, M_PAD) act as key/value candidates but are nulled by the kvc
    # indicator (V rows scaled to 0, denominator columns 0).
    R_PAD = max(128, _ceil_to(max(Ms[b] + tails[b] for b in range(B)), 16),
                M_PAD)

    NMT = M_PAD // 128
    xT_b, kvc_b, kvr_b, row_pos = [], [], [], []
    for b in range(B):
        xa = np.zeros((512, R_PAD), dtype=np.float32)
        xa[:, :Ms[b]] = x[b][kv_idx[b]].T
        xa[:, Ms[b]:Ms[b] + tails[b]] = x[b][tail_idx[b]].T
        xT_b.append(np.ascontiguousarray(xa.astype(bf16)))
        kvones = np.zeros(M_PAD, dtype=np.float32)
        kvones[:Ms[b]] = 1.0
        kvc_b.append(np.ascontiguousarray(kvones.reshape(NMT, 128).T))
        # kvc replicated into the 64 denominator columns per (m-tile, head)
        kvr = np.broadcast_to(kvc_b[b][:, :, None, None],
                              (128, NMT, 2, 64))
        kvr_b.append(np.ascontiguousarray(
            kvr.reshape(128, NMT * 128).astype(bf16)))
        # output row p of the device result corresponds to token row_pos[p]
        pos = np.concatenate([kv_idx[b], tail_idx[b]])
        row_pos.append(pos)

    scale = np.float32(_DH ** -0.5)
    in_maps = []
    for c in range(_CORES):
        b, g = divmod(c, 4)
        qc = slice(128 * g, 128 * g + 128)
        kc = slice(_INNER + 128 * g, _INNER + 128 * g + 128)
        vc = slice(2 * _INNER + 128 * g, 2 * _INNER + 128 * g + 128)
        w3 = np.ascontiguousarray(np.concatenate(
            [W_qkv[:, qc] * scale, W_qkv[:, kc], W_qkv[:, vc]],
            axis=1).astype(bf16))
        wo = np.ascontiguousarray(
            W_out[128 * g:128 * g + 128, :].astype(bf16))
        in_maps.append({"xT": xT_b[b], "w3": w3, "wo": wo,
                        "kvc": kvc_b[b], "kvr": kvr_b[b]})

    meta = dict(M_PAD=M_PAD, R_PAD=R_PAD, Ms=Ms, tails=tails,
                kv_idx=kv_idx, tail_idx=tail_idx, row_pos=row_pos,
                over_idx=over_idx)
    return in_maps, meta


def _assemble(results, meta, x, mask_np, W_qkv, W_out, b_out):
    B, N, _ = x.shape
    out = np.empty((B, N, _DIM), dtype=np.float32)
    Wv_full = W_qkv[:, 2 * _INNER:].astype(np.float32)
    for b in range(B):
        # constant output for fully-masked rows: uniform attention = mean(V)
        meanv = (x[b].mean(axis=0, dtype=np.float32) @ Wv_full)
        yconst = meanv @ W_out.astype(np.float32) + b_out
        out[b, :, :] = yconst[None, :]
        Mb, tb = meta["Ms"][b], meta["tails"][b]
        if Mb == 0:
            # no unmasked kv columns: every row fully masked -> uniform
            continue
        acc = None
        for g in range(4):
            yp = results[4 * b + g]["y"].astype(np.float32)
            acc = yp if acc is None else acc + yp
        out[b, meta["row_pos"][b], :] = acc[:Mb + tb] + b_out
        ov = meta["over_idx"][b]
        if len(ov):
            # host-peeled tail rows (queries only): exact attention over
            # the surviving kv columns.
            kv = meta["kv_idx"][b]
            scale = np.float32(_DH ** -0.5)
            q = (x[b][ov].astype(np.float32) @
                 W_qkv[:, :_INNER]).reshape(len(ov), _H, _DH)
            k = (x[b][kv].astype(np.float32) @
                 W_qkv[:, _INNER:2 * _INNER]).reshape(Mb, _H, _DH)
            v = (x[b][kv].astype(np.float32) @
                 W_qkv[:, 2 * _INNER:]).reshape(Mb, _H, _DH)
            s = np.einsum('ihd,jhd->hij', q, k) * scale
            s -= s.max(axis=-1, keepdims=True)
            p = np.exp(s)
            p /= p.sum(axis=-1, keepdims=True)
            o = np.einsum('hij,jhd->ihd', p, v).reshape(len(ov), _INNER)
            out[b, ov, :] = o @ W_out + b_out
    return out


_CACHE = {}


def _get_bass(R_PAD, M_PAD):
    key = (R_PAD, M_PAD)
    if key not in _CACHE:
        _CACHE[key] = build_bass(R_PAD, M_PAD)
    return _CACHE[key]


def run_spmd(in_maps, meta, trace=False, tmpdir=None, trace_cores=None):
    from concourse.bass_utils import run_bass_kernel_spmd

    nc = _get_bass(meta["R_PAD"], meta["M_PAD"])
    return run_bass_kernel_spmd(
        nc, in_maps, core_ids=list(range(_CORES)), trace=trace, tmpdir=tmpdir,
        trace_cores=trace_cores)


def kernel(x, mask_np, mask_bert, W_qkv, W_out, b_out):
    x = np.asarray(x)
    mask_np = np.asarray(mask_np)
    mask_bert = np.asarray(mask_bert)
    W_qkv = np.asarray(W_qkv, dtype=np.float32)
    W_out = np.asarray(W_out, dtype=np.float32)
    b_out = np.asarray(b_out, dtype=np.float32)

    in_maps, meta = _prep(x, mask_np, mask_bert, W_qkv, W_out)
    res = run_spmd(in_maps, meta)
    return _assemble(res.results, meta, x, mask_np, W_qkv, W_out, b_out)
